# revision 43
# baseline (speedup 1.0000x reference)
"""Trainium2 Bass kernel for nn_AnnsHNSW (retrieval kNN + anns pairing).

Full inputs: query [2,16,2048,64] f32, key [2,16,2048,64] f32, sample_size=64.
Output: (query_sort_idx [2,16,2048] i32, key_pick_idx [2,16,2048] i32).

Math note: the reference's QNF augmentation makes the per-query kNN ordering
identical to ordering by s = 2*r_q*(q.k) - |k_aug|^2 (descending), since the
augmented query norm term is constant per row and r_q > 0.  We compute s
directly in PSUM via a K=65 matmul (row 64 of lhsT = 1, row 64 of rhs = -k2).

Sharding: 32 (b,h) slices -> 8 cores x 4 slices, embarrassingly parallel.
"""

import os

import numpy as np

B, H, NQ, NK, D = 2, 16, 2048, 2048, 64
SAMPLE = 64
N_CORES = 8
SL = (B * H) // N_CORES  # slices per core

NEG_BIG = -1.0e30


def build_bass(n_slices=SL, nq=NQ, nk=NK, d=D, sample=SAMPLE, split_waits=True, debug=False):
    import concourse.bass as bass
    import concourse.mybir as mybir
    from concourse.tile import TileContext
    from concourse.masks import make_identity

    f32 = mybir.dt.float32
    i32 = mybir.dt.int32
    u32 = mybir.dt.uint32
    AF = mybir.ActivationFunctionType
    ALU = mybir.AluOpType
    AX = mybir.AxisListType

    nqt = nq // 128          # q tiles per slice
    nkt = nk // 128          # k tiles per slice
    nch = nk // 512          # 512-wide psum chunks
    npick = nq // sample     # picked queries per slice (32)
    assert npick * n_slices <= 128

    nc = bass.Bass()
    q_in = nc.declare_dram_parameter("query", [n_slices, nq, d], f32, isOutput=False)
    k_in = nc.declare_dram_parameter("key", [n_slices, nk, d], f32, isOutput=False)
    qiota_f_in = nc.declare_dram_parameter("qiota_f", [128, nqt], f32, isOutput=False)
    qiota_i_in = nc.declare_dram_parameter("qiota_i", [128, nqt], i32, isOutput=False)
    negk2_in = nc.declare_dram_parameter("negk2", [n_slices, nk], f32, isOutput=False)
    r2_in = nc.declare_dram_parameter("r2in", [n_slices, 128, nqt], f32, isOutput=False)
    qsi_out = nc.declare_dram_parameter("qsi", [n_slices * nq, 1], i32, isOutput=True)
    kpi_out = nc.declare_dram_parameter("kpi", [n_slices, nq], i32, isOutput=True)

    if debug:
        dbg_lab = nc.declare_dram_parameter("dbg_lab", [n_slices, 128, nq // 128], f32, isOutput=True)
        dbg_rank = nc.declare_dram_parameter("dbg_rank", [n_slices, 128, nq // 128], f32, isOutput=True)
        dbg_cb = nc.declare_dram_parameter("dbg_cb", [n_slices, 128, nq], f32, isOutput=True)

    qs_drams = [
        nc.dram_tensor(f"qs_dram{s}", [nq, d], f32) for s in range(n_slices)
    ]
    crow_dram = nc.dram_tensor("crow_dram", [n_slices, nq], f32)

    with TileContext(nc) as tc:
        with (
            tc.tile_pool(name="const", bufs=1) as constp,
            tc.tile_pool(name="io", bufs=3) as iop,
            tc.tile_pool(name="qtiles", bufs=2) as qtilep,
            tc.tile_pool(name="ktp", bufs=n_slices) as ktp,
            tc.tile_pool(name="qtp", bufs=2) as qtp,
            tc.tile_pool(name="cbp", bufs=2) as cbp,
            tc.tile_pool(name="scrp", bufs=2) as scrp,
            tc.tile_pool(name="smallp", bufs=2) as smallp,
            tc.tile_pool(name="v8p", bufs=4) as v8p,
            tc.tile_pool(name="accp", bufs=3) as accp,
            tc.tile_pool(name="pickp", bufs=1) as pickp,
            tc.tile_pool(name="ps_scoresA", bufs=1, space="PSUM") as ps_scoresA,
            tc.tile_pool(name="ps_scoresB", bufs=1, space="PSUM") as ps_scoresB,
            tc.tile_pool(name="ps_trp", bufs=1, space="PSUM") as ps_trp,
            tc.tile_pool(name="ps_cbp", bufs=1, space="PSUM") as ps_cbp,
            tc.tile_pool(name="ps_ppp", bufs=1, space="PSUM") as ps_ppp,
        ):
            # ---- constants ----
            ident = constp.tile([128, 128], f32, tag="ident")
            make_identity(nc, ident[:])
            ones1 = constp.tile([1, 128], f32, tag="ones1")
            nc.vector.memset(ones1[:], 1.0)
            qiota_f = constp.tile([128, nqt], f32, tag="qiota_f")
            nc.sync.dma_start(qiota_f[:], qiota_f_in[:])
            qiota_i = constp.tile([128, nqt], i32, tag="qiota_i")
            nc.sync.dma_start(qiota_i[:], qiota_i_in[:])

            # persistent PSUM tiles: single tensors so slot-reuse WAW stays in
            # PE program order (fp32 matmuls can carry only ONE sync wait)
            ps_scA = ps_scoresA.tile([128, nk // 2], f32, tag="scoresA")
            ps_scB = ps_scoresB.tile([128, nk // 2], f32, tag="scoresB")
            ps_tr = ps_trp.tile([128, 128], f32, tag="ps_tr")
            ps_cb = ps_cbp.tile([128, 512], f32, tag="ps_cb")

            # dummy PE matmul so later PE ops don't re-wait on ident (gpsimd)
            nc.tensor.matmul(ps_tr[:], lhsT=ident[:], rhs=ident[:], start=True, stop=True)
            dscrap = smallp.tile([1, 1], f32, tag="dscrap")
            nc.vector.tensor_copy(dscrap[:], ps_tr[0:1, 0:1])

            # picked-phase tiles (assembled across slices)
            nused = npick * n_slices
            pqt = pickp.tile([d + 1, nused], f32, tag="pqt")
            nc.vector.memset(pqt[d : d + 1, :], 1.0)
            topidx = pickp.tile([nused, sample], i32, tag="topidx")
            psc = pickp.tile([nused, nk], f32, tag="psc")
            ps_pp = ps_ppp.tile([nused, 512], f32, tag="ps_pp")

            kts = []

            qts = {}
            labfs = {}

            def preproc(s):
                # ================= per-key preproc =================
                kt = ktp.tile([d + 1, nk], f32, tag="kt")
                kts.append(kt)
                kbig = iop.tile([128, nkt * d], f32, tag="kbig")
                nc.sync.dma_start(
                    kbig[:].rearrange("p (t d) -> p t d", d=d),
                    k_in[s].rearrange("(t p) d -> p t d", p=128),
                )
                for t in range(nkt):
                    nc.tensor.matmul(ps_tr[0:d, :], lhsT=kbig[:, t * d : (t + 1) * d], rhs=ident[:], start=True, stop=True)
                    nc.scalar.copy(kt[0:d, t * 128 : (t + 1) * 128], ps_tr[0:d, :])

                # row 64 = -|k_aug|^2, computed on host (bit-matches reference)
                k2stage = smallp.tile([1, nk], f32, tag="k2stage")
                nc.sync.dma_start(k2stage[:], negk2_in[s : s + 1, :])
                nc.vector.tensor_copy(kt[d : d + 1, :], k2stage[:])

                # ================= per-query preproc =================
                qt = qtp.tile([d + 1, nq], f32, tag="qt")
                nc.vector.memset(qt[d : d + 1, :], 1.0)
                qbig = qtilep.tile([128, nqt * d], f32, tag="qbig")
                nc.sync.dma_start(
                    qbig[:].rearrange("p (t d) -> p t d", d=d),
                    q_in[s].rearrange("(t p) d -> p t d", p=128),
                )
                # r2 = 2*key_norm_max/query_norm, computed on host
                r2 = smallp.tile([128, nqt], f32, tag="r2")
                nc.sync.dma_start(r2[:], r2_in[s])

                for t in range(nqt):
                    qs = iop.tile([128, d], f32, tag="qs")
                    nc.vector.tensor_scalar(
                        qs[:], qbig[:, t * d : (t + 1) * d],
                        r2[:, t : t + 1], None, op0=ALU.mult,
                    )
                    nc.sync.dma_start(
                        qs_drams[s][t * 128 : (t + 1) * 128, :], qs[:]
                    )
                    nc.tensor.matmul(ps_tr[0:d, :], lhsT=qs[:], rhs=ident[:], start=True, stop=True)
                    nc.scalar.copy(qt[0:d, t * 128 : (t + 1) * 128], ps_tr[0:d, :])

                qts[s] = qt

            def labels(s):
                kt = kts[s]
                qt = qts[s]
                # ================= scores + labels =================
                # two half-row PSUM tiles: PE fills half B while DVE scans A
                half = nk // 2
                hch = half // 512
                va8 = accp.tile([128, nqt * 8], f32, tag="va8")
                vb8 = accp.tile([128, nqt * 8], f32, tag="vb8")
                ia8 = accp.tile([128, nqt * 8], u32, tag="ia8")
                ib8 = accp.tile([128, nqt * 8], u32, tag="ib8")
                for t in range(nqt):
                    for n in range(hch):
                        nc.tensor.matmul(
                            ps_scA[:, n * 512 : (n + 1) * 512],
                            lhsT=qt[:, t * 128 : (t + 1) * 128],
                            rhs=kt[:, n * 512 : (n + 1) * 512],
                            start=True,
                            stop=True,
                        )
                    for n in range(hch):
                        nc.tensor.matmul(
                            ps_scB[:, n * 512 : (n + 1) * 512],
                            lhsT=qt[:, t * 128 : (t + 1) * 128],
                            rhs=kt[:, half + n * 512 : half + (n + 1) * 512],
                            start=True,
                            stop=True,
                        )
                    v8a = va8[:, t * 8 : (t + 1) * 8]
                    nc.vector.max(out=v8a, in_=ps_scA[:])
                    i8a = ia8[:, t * 8 : (t + 1) * 8]
                    nc.vector.max_index(out=i8a, in_max=v8a, in_values=ps_scA[:])
                    v8b = vb8[:, t * 8 : (t + 1) * 8]
                    nc.vector.max(out=v8b, in_=ps_scB[:])
                    i8b = ib8[:, t * 8 : (t + 1) * 8]
                    nc.vector.max_index(out=i8b, in_max=v8b, in_values=ps_scB[:])
                # label = (va >= vb) ? ia : ib + half   (ties -> A = lower idx,
                # matching lax.top_k; within-half first-match is exact)
                # strided views of column 0 of each 8-wide result group
                va_s = va8[:].rearrange("p (t e) -> p t e", e=8)[:, :, 0]
                vb_s = vb8[:].rearrange("p (t e) -> p t e", e=8)[:, :, 0]
                ia = smallp.tile([128, nqt], f32, tag="ia")
                nc.vector.tensor_copy(ia[:], ia8[:].rearrange("p (t e) -> p t e", e=8)[:, :, 0])
                ib = smallp.tile([128, nqt], f32, tag="ib")
                nc.vector.tensor_copy(ib[:], ib8[:].rearrange("p (t e) -> p t e", e=8)[:, :, 0])
                labf = smallp.tile([128, nqt], f32, tag="labf")
                cge = smallp.tile([128, nqt], f32, tag="cge")
                nc.vector.tensor_tensor(cge[:], va_s, vb_s, op=ALU.is_ge)
                t1 = smallp.tile([128, nqt], f32, tag="t1")
                nc.vector.tensor_scalar(t1[:], ib[:], float(half), None, op0=ALU.add)
                nc.vector.tensor_tensor(t1[:], t1[:], ia[:], op=ALU.subtract)
                nc.vector.tensor_tensor(t1[:], t1[:], cge[:], op=ALU.mult)
                nc.vector.tensor_tensor(labf[:], ib[:], t1[:], op=ALU.subtract)
                nc.vector.tensor_scalar(labf[:], labf[:], float(half), None, op0=ALU.add)

                labfs[s] = labf

            def post(s):
                labf = labfs[s]
                # ================= rank (stable argsort of labels) =================
                c = smallp.tile([128, nqt], f32, tag="c")
                nc.vector.tensor_scalar(c[:], labf[:], float(nq), None, op0=ALU.mult)
                nc.vector.tensor_tensor(c[:], c[:], qiota_f[:], op=ALU.add)
                nc.tensor.matmul(ps_tr[0:nqt, :], lhsT=c[:], rhs=ident[:], start=True, stop=True)
                ct = smallp.tile([nqt, 128], f32, tag="ct")
                nc.vector.tensor_copy(ct[:], ps_tr[0:nqt, :])
                nc.sync.dma_start(
                    crow_dram[s].rearrange("(t p) -> t p", t=nqt), ct[:]
                )
                crow_st = smallp.tile([1, nq], f32, tag="crow_st")
                nc.sync.dma_start(crow_st[:], crow_dram[s : s + 1, :])
                crow = smallp.tile([1, nq], f32, tag="crow")
                nc.scalar.copy(crow[:], crow_st[:])

                cb = cbp.tile([128, nq], f32, tag="cb")
                for n in range(nq // 512):
                    nc.tensor.matmul(
                        ps_cb[:], lhsT=ones1[:], rhs=crow[:, n * 512 : (n + 1) * 512],
                        start=True, stop=True,
                    )
                    nc.scalar.copy(cb[:, n * 512 : (n + 1) * 512], ps_cb[:])

                acc = smallp.tile([128, nqt], f32, tag="acc")
                rscr = scrp.tile([128, nq], f32, tag="rscr")
                rankf = smallp.tile([128, nqt], f32, tag="rankf")
                ranki = smallp.tile([128, nqt], i32, tag="ranki")
                # last slice: no next-slice scans to run, so give half the rank
                # columns to the otherwise-idle DVE (count of c_j < c_own is
                # the rank directly; c values are distinct by construction)
                use_dve = False  # TENSOR_TENSOR_REDUCE hits "ISA wrong
                # length" in this walrus build (see probes); keep rank on ACT
                for t in range(nqt):
                    if use_dve and t % 2 == 1:
                        rscr2 = scrp.tile([128, nq], f32, tag="rscr2")
                        nc.vector.tensor_tensor_reduce(
                            out=rscr2[:], in0=cb[:],
                            in1=c[:, t : t + 1].to_broadcast([128, nq]),
                            scale=1.0, scalar=0.0,
                            op0=ALU.is_lt, op1=ALU.add,
                            accum_out=rankf[:, t : t + 1],
                        )
                    else:
                        nc.scalar.activation(
                            rscr[:], cb[:], AF.Sign,
                            bias=c[:, t : t + 1], scale=-1.0,
                            accum_out=acc[:, t : t + 1],
                        )
                        nc.vector.tensor_scalar(
                            rankf[:, t : t + 1], acc[:, t : t + 1],
                            float(nq - 1), 0.5, op0=ALU.add, op1=ALU.mult,
                        )
                    nc.vector.tensor_copy(ranki[:, t : t + 1], rankf[:, t : t + 1])
                    nc.gpsimd.indirect_dma_start(
                        out=qsi_out[:],
                        out_offset=bass.IndirectOffsetOnAxis(
                            ap=ranki[:, t : t + 1], axis=0
                        ),
                        in_=qiota_i[:, t : t + 1],
                        in_offset=None,
                        element_offset=s * nq,
                    )
                if debug:
                    nc.sync.dma_start(dbg_lab[s], labf[:])
                    nc.sync.dma_start(dbg_cb[s], cb[:])
                if debug:
                    nc.sync.dma_start(dbg_rank[s], rankf[:])

                # ================= picked queries =================
                picked = pickp.tile([npick, 1], i32, tag="picked")
                nc.sync.dma_start(
                    picked[:],
                    qsi_out[:].rearrange(
                        "(s j k) one -> s j (k one)", s=n_slices, k=sample
                    )[s, :, 0:1],
                )
                pq = pickp.tile([npick, d], f32, tag="pq")
                nc.gpsimd.indirect_dma_start(
                    out=pq[:],
                    out_offset=None,
                    in_=qs_drams[s][:],
                    in_offset=bass.IndirectOffsetOnAxis(ap=picked[:], axis=0),
                )
                pq2 = pickp.tile([npick, d], f32, tag="pq2")
                nc.scalar.copy(pq2[:], pq[:])
                nc.tensor.matmul(ps_tr[0:d, 0:npick], lhsT=pq2[:], rhs=ident[0:npick, 0:npick], start=True, stop=True)
                nc.vector.tensor_copy(pqt[0:d, s * npick : (s + 1) * npick], ps_tr[0:d, 0:npick])
                # picked scores for this slice now (overlaps later slices)
                for n in range(nch):
                    nc.tensor.matmul(
                        ps_pp[s * npick : (s + 1) * npick, :],
                        lhsT=pqt[:, s * npick : (s + 1) * npick],
                        rhs=kts[s][:, n * 512 : (n + 1) * 512],
                        start=True,
                        stop=True,
                        tile_position=(0, s * npick),
                    )
                    nc.scalar.copy(
                        psc[s * npick : (s + 1) * npick, n * 512 : (n + 1) * 512],
                        ps_pp[s * npick : (s + 1) * npick, :],
                    )


            preproc(0)
            for s in range(n_slices):
                labels(s)
                if s + 1 < n_slices:
                    preproc(s + 1)
                post(s)

            # ================= picked scores + top-64 =================

            for r in range(sample // 8):
                pv8 = v8p.tile([nused, 8], f32, tag="pv8")
                nc.vector.max(out=pv8[:], in_=psc[:])
                nc.vector.max_index(
                    out=topidx[:, r * 8 : (r + 1) * 8].bitcast(u32),
                    in_max=pv8[:], in_values=psc[:],
                )
                if r < sample // 8 - 1:
                    nc.vector.match_replace(
                        out=psc[:], in_to_replace=pv8[:], in_values=psc[:],
                        imm_value=NEG_BIG,
                    )

            for s in range(n_slices):
                nc.sync.dma_start(
                    kpi_out[s].rearrange("(j k) -> j k", k=sample),
                    topidx[s * npick : (s + 1) * npick, :],
                )

    if split_waits:
        _split_multi_waits(nc, mybir)
    return nc


def _split_multi_waits(nc, mybir):
    """This walrus build accepts only ONE sync-wait per compute/DMA
    instruction.  Move extra waits onto same-engine NoOps inserted before the
    offending instruction (engine executes NoOp waits first, program order)."""
    n = 0
    for f in nc.m.functions:
        for blk in f.blocks:
            out = []
            for inst in blk.instructions:
                si = getattr(inst, "sync_info", None)
                if si is not None and len(si.on_wait) > 1:
                    waits = list(si.on_wait)
                    for w in waits[:-1]:
                        nop = mybir.InstNoOp(
                            name=f"I-wsplit-{n}", ins=[], outs=[],
                            text_hint="wsplit",
                        )
                        n += 1
                        nop.engine = inst.engine
                        nop.sync_info = mybir.SyncInfo(on_wait=[w], on_update=[])
                        out.append(nop)
                    inst.sync_info = mybir.SyncInfo(
                        on_wait=[waits[-1]], on_update=list(si.on_update)
                    )
                out.append(inst)
            blk.instructions = out
    return nc


_BUILT = {}
LAST_RESULTS = None


def _get_nc(key=(SL, NQ, NK, D, SAMPLE)):
    if key not in _BUILT:
        _BUILT[key] = build_bass(*key)
    return _BUILT[key]


def make_iota(nqt=NQ // 128):
    # qiota[p, t] = t*128 + p
    p = np.arange(128, dtype=np.int64)[:, None]
    t = np.arange(nqt, dtype=np.int64)[None, :]
    v = (t * 128 + p)
    return v.astype(np.float32), v.astype(np.int32)


def qnf_factors(q, k, nq=NQ, nk=NK):
    """Host-side QNF factors (correctly-rounded fp32, matching the CPU
    reference's sqrt chain): negk2 [S, nk] and r2 [S, 128, nq//128]."""
    n = q.shape[0]
    negk2 = np.empty((n, nk), np.float32)
    r2t = np.empty((n, 128, nq // 128), np.float32)
    for s in range(n):
        kk, qq = k[s], q[s]
        s2k = np.einsum("kd,kd->k", kk, kk).astype(np.float32)
        norm = np.sqrt(s2k)
        mx = norm.max()
        extra = np.sqrt(np.maximum(mx * mx - norm * norm, np.float32(0.0)))
        negk2[s] = -(s2k + extra * extra)
        qn = np.maximum(
            np.sqrt(np.einsum("qd,qd->q", qq, qq).astype(np.float32)),
            np.float32(1e-6),
        )
        r2 = (np.float32(2.0) * mx) / qn
        r2t[s] = r2.reshape(nq // 128, 128).T
    return negk2, r2t


def kernel(query, key, sample_size=SAMPLE):
    from concourse.bass_utils import run_bass_kernel_spmd

    q = np.ascontiguousarray(np.asarray(query, dtype=np.float32)).reshape(B * H, NQ, D)
    k = np.ascontiguousarray(np.asarray(key, dtype=np.float32)).reshape(B * H, NK, D)
    iota_f, iota_i = make_iota()
    negk2_all, r2_all = qnf_factors(q, k)

    in_maps = []
    for c in range(N_CORES):
        in_maps.append(
            {
                "query": np.ascontiguousarray(q[c * SL : (c + 1) * SL]),
                "key": np.ascontiguousarray(k[c * SL : (c + 1) * SL]),
                "qiota_f": iota_f,
                "qiota_i": iota_i,
                "negk2": np.ascontiguousarray(negk2_all[c * SL : (c + 1) * SL]),
                "r2in": np.ascontiguousarray(r2_all[c * SL : (c + 1) * SL]),
            }
        )

    nc = _get_nc()
    trace = bool(os.environ.get("ANNS_TRACE"))
    res = run_bass_kernel_spmd(
        nc, in_maps, core_ids=list(range(N_CORES)), trace=trace
    )
    global LAST_RESULTS
    LAST_RESULTS = res
    qsi = np.concatenate(
        [res.results[i]["qsi"].reshape(SL, NQ) for i in range(N_CORES)], axis=0
    ).reshape(B, H, NQ)
    kpi = np.concatenate(
        [res.results[i]["kpi"].reshape(SL, NQ) for i in range(N_CORES)], axis=0
    ).reshape(B, H, NQ)
    return qsi.astype(np.int32), kpi.astype(np.int32)


if __name__ == "__main__":
    import jax

    jax.config.update("jax_platforms", "cpu")
    rng = np.random.default_rng(0)
    q = rng.normal(size=(B, H, NQ, D)).astype(np.float32)
    k = rng.normal(size=(B, H, NK, D)).astype(np.float32)
    out = kernel(q, k, SAMPLE)
    print([o.shape for o in out])
